# revision 9
# baseline (speedup 1.0000x reference)
"""Trainium2 Bass kernel for nn_DirectMultiStepModel (2-layer graph-GRU + big Linear + softmax).

Self-contained: takes FULL inputs, shards nodes across 8 NeuronCores internally,
runs a single SPMD NEFF with on-device collectives, returns the FULL (1, 100) output.

Strategy:
  - Host: materialize the normalized adjacency as a dense padded matrix M
    (N=10000 -> 10240), shard destination nodes across cores (1280 each).
  - Device per core: GRU1 (feature-major layout) -> transpose -> per-t-block
    AllGather of h1 -> dense aggregation matmul (M^T against gathered H) with
    fused bias+ReLU -> GRU2 (gi2 computed on the fly from streamed agg1) ->
    AllGather h2[T-1] -> agg2 -> column-sharded matvec against lin_W ->
    AllReduce partial logits -> softmax.
"""
import sys
import types
import numpy as np
import ml_dtypes

import concourse.bass as bass
import concourse.bacc as bacc
import concourse.mybir as mybir
import concourse.tile as tile
from concourse.bass_utils import run_bass_kernel_spmd

BF16 = ml_dtypes.bfloat16
F32 = mybir.dt.float32
BF = mybir.dt.bfloat16
P = 128


def _install_ntff_hook():
    """Register the NTFF profile hook the agent image's antenv lacks (no-op if present)."""
    try:
        import antenv.axon_hooks  # noqa: F401
        return
    except ImportError:
        pass
    try:
        import trn_agent_boot.trn_boot as tb
        hooks = types.ModuleType("antenv.axon_hooks")
        _h = [None]
        hooks.set_axon_ntff_profile_hook = lambda h: _h.__setitem__(0, h)
        hooks.get_axon_ntff_profile_hook = lambda: _h[0]
        sys.modules["antenv.axon_hooks"] = hooks
        import antenv
        antenv.axon_hooks = hooks
        hook = tb._ntff_profile_via_ctypes('/opt/axon/libaxon_pjrt.so')
        if hook is not None:
            hooks.set_axon_ntff_profile_hook(hook)
    except Exception:
        pass


class Cfg:
    def __init__(self, T=24, N=10000, DIN=128, H1=256, H2=128, OUT=100, NC=8, TB=3):
        self.T, self.N, self.DIN, self.H1, self.H2, self.OUT, self.NC = T, N, DIN, H1, H2, OUT, NC
        self.NOWN = -(-N // (NC * P)) * P          # per-core padded node count
        self.NPAD = self.NOWN * NC                 # total padded nodes
        self.NT = self.NOWN // P                   # own node tiles
        self.CT = self.NPAD // P                   # contraction tiles
        self.PS1 = H1 // P                         # h1 feature partition-tiles
        self.PS2 = H2 // P
        self.G1, self.G2 = 3 * H1, 3 * H2
        self.TB = TB                               # timesteps per AllGather block
        assert T % TB == 0
        self.NB = T // TB
        self.HALF = self.NOWN // 2                 # own-node split for Mt residency
        assert self.HALF % 2 == 0


def fchunks(total, maxf=512):
    out, off = [], 0
    while off < total:
        fl = min(maxf, total - off)
        out.append((off, fl))
        off += fl
    return out


def build(cfg: Cfg):
    """Build + compile the SPMD kernel. Returns the compiled Bacc."""
    c = cfg
    nc = bacc.Bacc("TRN2", target_bir_lowering=False, debug=False, num_devices=c.NC)

    # ---- kernel I/O ----
    xT = nc.dram_tensor("xT", [c.T, c.DIN, c.NOWN], BF, kind="ExternalInput").ap()
    wih1T = nc.dram_tensor("wih1T", [c.DIN, c.G1], BF, kind="ExternalInput").ap()
    whh1T = nc.dram_tensor("whh1T", [c.H1, c.G1], BF, kind="ExternalInput").ap()
    wih2T = nc.dram_tensor("wih2T", [c.H1, c.G2], BF, kind="ExternalInput").ap()
    whh2T = nc.dram_tensor("whh2T", [c.H2, c.G2], BF, kind="ExternalInput").ap()
    b1_rz = nc.dram_tensor("b1_rz", [2 * c.H1, 1], F32, kind="ExternalInput").ap()
    b1_hn = nc.dram_tensor("b1_hn", [c.H1, 1], F32, kind="ExternalInput").ap()
    b1_in = nc.dram_tensor("b1_in", [c.H1, 1], F32, kind="ExternalInput").ap()
    b2_rz = nc.dram_tensor("b2_rz", [2 * c.H2, 1], F32, kind="ExternalInput").ap()
    b2_hn = nc.dram_tensor("b2_hn", [c.H2, 1], F32, kind="ExternalInput").ap()
    b2_in = nc.dram_tensor("b2_in", [c.H2, 1], F32, kind="ExternalInput").ap()
    cb1 = nc.dram_tensor("cb1", [c.H1, 1], F32, kind="ExternalInput").ap()
    cb2 = nc.dram_tensor("cb2", [c.H2, 1], F32, kind="ExternalInput").ap()
    mT = nc.dram_tensor("mT", [c.NPAD, c.NOWN], BF, kind="ExternalInput").ap()
    linW4 = nc.dram_tensor("linW4", [c.NT, c.H2, P, c.OUT], BF, kind="ExternalInput").ap()
    linb = nc.dram_tensor("linb", [1, c.OUT], F32, kind="ExternalInput").ap()
    ident = nc.dram_tensor("ident", [P, P], BF, kind="ExternalInput").ap()
    out = nc.dram_tensor("out", [1, c.OUT], F32, kind="ExternalOutput").ap()

    rg = [list(range(c.NC))]
    AG_ROWS = c.TB * c.PS1 * c.NOWN  # rows per core per t-block in the h1 all-gather

    with tile.TileContext(nc) as tc:
        with tc.tile_pool(name="dram", bufs=1, space="DRAM") as dram:
            ag_in = dram.tile([c.NB, AG_ROWS, P], BF)
            ag_outs = [dram.tile([AG_ROWS * c.NC, P], BF, addr_space="Shared",
                                 name=f"ag_out{i}") for i in range(c.NB)]
            agg1 = dram.tile([c.T, c.PS1, P, c.NOWN], BF)
            ag2_in = dram.tile([c.NOWN, P], BF)
            ag2_out = dram.tile([c.NOWN * c.NC, P], BF, addr_space="Shared")
            ar_in = dram.tile([1, c.OUT], F32)
            ar_out = dram.tile([1, c.OUT], F32, addr_space="Shared")

            # ---- constants in SBUF (live for the whole kernel) ----
            with tc.tile_pool(name="const", bufs=1) as cpool:
                wih1_sb = cpool.tile([P, c.G1], BF)
                nc.sync.dma_start(wih1_sb[:], wih1T[:])
                whh1_sb = cpool.tile([P, c.PS1 * c.G1], BF)
                for cc in range(c.PS1):
                    nc.sync.dma_start(whh1_sb[:, cc * c.G1:(cc + 1) * c.G1],
                                      whh1T[cc * P:(cc + 1) * P, :])
                wih2_sb = cpool.tile([P, c.PS1 * c.G2], BF)
                for cc in range(c.PS1):
                    nc.sync.dma_start(wih2_sb[:, cc * c.G2:(cc + 1) * c.G2],
                                      wih2T[cc * P:(cc + 1) * P, :])
                whh2_sb = cpool.tile([P, c.PS2 * c.G2], BF)
                for cc in range(c.PS2):
                    nc.sync.dma_start(whh2_sb[:, cc * c.G2:(cc + 1) * c.G2],
                                      whh2T[cc * P:(cc + 1) * P, :])
                ident_sb = cpool.tile([P, P], BF)
                nc.sync.dma_start(ident_sb[:], ident[:])

                def bias_tile(src, n):
                    t = cpool.tile([P, n // P], F32, name=f"b_{src.tensor.name}")
                    for i in range(n // P):
                        nc.sync.dma_start(t[:, i:i + 1], src[i * P:(i + 1) * P, :])
                    return t
                b1rz_sb = bias_tile(b1_rz, 2 * c.H1)
                b1hn_sb = bias_tile(b1_hn, c.H1)
                b1in_sb = bias_tile(b1_in, c.H1)
                b2rz_sb = bias_tile(b2_rz, 2 * c.H2)
                b2hn_sb = bias_tile(b2_hn, c.H2)
                b2in_sb = bias_tile(b2_in, c.H2)
                cb1_sb = bias_tile(cb1, c.H1)
                cb2_sb = bias_tile(cb2, c.H2)
                linb_sb = cpool.tile([1, c.OUT], F32)
                nc.sync.dma_start(linb_sb[:], linb[:])

                FC = fchunks(c.NOWN)           # node chunks (<=512) for gate matmuls
                Sig = mybir.ActivationFunctionType.Sigmoid
                Tanh = mybir.ActivationFunctionType.Tanh
                Iden = mybir.ActivationFunctionType.Identity
                Relu = mybir.ActivationFunctionType.Relu
                Exp = mybir.ActivationFunctionType.Exp
                Copy = mybir.ActivationFunctionType.Copy

                def gru_step(xsrc, x_ct, wih_sb, whh_sb, brz, bhn, bin_, h_sb, ps_n,
                             psum, work, G):
                    """One GRU step, feature-major. xsrc: list of x rhs slices (one per
                    contraction tile of the input); h_sb: state tile (P, ps_n*NOWN) bf16
                    updated in place; ps_n: feature tiles of the hidden dim; G: gate count."""
                    NOWN = c.NOWN
                    rz = work.tile([P, 2 * ps_n * NOWN], BF, tag="rz")
                    nsb = work.tile([P, ps_n * NOWN], BF, tag="nsb")
                    insb = work.tile([P, ps_n * NOWN], BF, tag="insb")
                    hnsb = work.tile([P, ps_n * NOWN], BF, tag="hnsb")
                    # r,z gates: fused x-part + h-part matmul, sigmoid evict
                    for g in range(2 * ps_n):
                        for (fo, fl) in FC:
                            pt = psum.tile([P, fl], F32, tag="ps_g")
                            for i, xs in enumerate(x_ct):
                                nc.tensor.matmul(pt[:], wih_sb[:, xs * G + g * P:xs * G + g * P + P],
                                                 xsrc[i][:, fo:fo + fl],
                                                 start=(i == 0), stop=False)
                            for cc in range(ps_n):
                                nc.tensor.matmul(pt[:], whh_sb[:, cc * G + g * P:cc * G + g * P + P],
                                                 h_sb[:, cc * NOWN + fo:cc * NOWN + fo + fl],
                                                 start=False, stop=(cc == ps_n - 1))
                            nc.scalar.activation(rz[:, g * NOWN + fo:g * NOWN + fo + fl], pt[:],
                                                 Sig, bias=brz[:, g:g + 1])
                    # i_n (x part only) and h_n (h part only)
                    for g2 in range(ps_n):
                        gofs = (2 * ps_n + g2) * P
                        for (fo, fl) in FC:
                            pi = psum.tile([P, fl], F32, tag="ps_g")
                            for i, xs in enumerate(x_ct):
                                nc.tensor.matmul(pi[:], wih_sb[:, xs * G + gofs:xs * G + gofs + P],
                                                 xsrc[i][:, fo:fo + fl],
                                                 start=(i == 0), stop=(i == len(x_ct) - 1))
                            nc.vector.tensor_scalar_add(
                                insb[:, g2 * NOWN + fo:g2 * NOWN + fo + fl], pi[:],
                                bin_[:, g2:g2 + 1])
                            ph = psum.tile([P, fl], F32, tag="ps_g")
                            for cc in range(ps_n):
                                nc.tensor.matmul(ph[:], whh_sb[:, cc * G + gofs:cc * G + gofs + P],
                                                 h_sb[:, cc * NOWN + fo:cc * NOWN + fo + fl],
                                                 start=(cc == 0), stop=(cc == ps_n - 1))
                            nc.scalar.activation(hnsb[:, g2 * NOWN + fo:g2 * NOWN + fo + fl],
                                                 ph[:], Iden, bias=bhn[:, g2:g2 + 1])
                    # gate math: n = tanh(i_n + r*hn); h' = n + z*(h-n)
                    for g2 in range(ps_n):
                        sl = slice(g2 * NOWN, (g2 + 1) * NOWN)
                        r_sl = slice(g2 * NOWN, (g2 + 1) * NOWN)
                        z_sl = slice((ps_n + g2) * NOWN, (ps_n + g2 + 1) * NOWN)
                        nc.vector.tensor_mul(hnsb[:, sl], rz[:, r_sl], hnsb[:, sl])
                        nc.vector.tensor_add(hnsb[:, sl], hnsb[:, sl], insb[:, sl])
                        nc.scalar.activation(nsb[:, sl], hnsb[:, sl], Tanh)
                        nc.gpsimd.tensor_sub(hnsb[:, sl], h_sb[:, sl], nsb[:, sl])
                        nc.vector.tensor_mul(hnsb[:, sl], rz[:, z_sl], hnsb[:, sl])
                        nc.vector.tensor_add(h_sb[:, sl], nsb[:, sl], hnsb[:, sl])

                # ================= Phase 1: GRU layer 1 (+ transpose + AllGather) ====
                with tc.tile_pool(name="p1", bufs=1) as p1, \
                     tc.tile_pool(name="p1x", bufs=3) as p1x, \
                     tc.tile_pool(name="p1w", bufs=2) as p1w, \
                     tc.tile_pool(name="ps1", bufs=5, space="PSUM") as ps1, \
                     tc.tile_pool(name="ps1t", bufs=2, space="PSUM") as ps1t, \
                     tc.tile_pool(name="p1s", bufs=3) as p1s:
                    h1 = p1.tile([P, c.PS1 * c.NOWN], BF)
                    nc.vector.memset(h1[:], 0.0)
                    for t in range(c.T):
                        xt = p1x.tile([P, c.NOWN], BF, tag="xt")
                        nc.sync.dma_start(xt[:], xT[t])
                        gru_step([xt], [0], wih1_sb, whh1_sb, b1rz_sb, b1hn_sb, b1in_sb,
                                 h1, c.PS1, ps1, p1w, c.G1)
                        # transpose h1_t to node-major and stage into the AG input
                        tb, tt = t // c.TB, t % c.TB
                        for ps in range(c.PS1):
                            stg = p1s.tile([P, c.NOWN], BF, tag="stg")
                            for nt in range(c.NT):
                                pt = ps1t.tile([P, P], BF, tag="ps_t")
                                nc.tensor.transpose(
                                    pt[:], h1[:, ps * c.NOWN + nt * P:ps * c.NOWN + (nt + 1) * P],
                                    ident_sb[:])
                                nc.vector.tensor_copy(stg[:, nt * P:(nt + 1) * P], pt[:])
                            ro = (tt * c.PS1 + ps) * c.NOWN
                            nc.sync.dma_start(
                                ag_in[tb, ro:ro + c.NOWN, :].rearrange("(nt p) f -> p nt f", p=P),
                                stg[:].rearrange("p (nt f) -> p nt f", f=P))
                        if tt == c.TB - 1:
                            nc.gpsimd.collective_compute(
                                "AllGather", mybir.AluOpType.bypass, replica_groups=rg,
                                ins=[ag_in[tb].opt()], outs=[ag_outs[tb].opt()])

                # ================= Phase 2: aggregation agg1 = M @ h1 (two halves) ===
                FCH = fchunks(c.HALF, 320)
                for half in range(2):
                    ho = half * c.HALF
                    with tc.tile_pool(name="mtp", bufs=1) as mtp, \
                         tc.tile_pool(name="hstp", bufs=2) as hstp, \
                         tc.tile_pool(name="aps", bufs=4, space="PSUM") as aps, \
                         tc.tile_pool(name="astg", bufs=3) as astg:
                        mt_sb = mtp.tile([P, c.CT * c.HALF], BF)
                        for ct in range(c.CT):
                            nc.sync.dma_start(
                                mt_sb[:, ct * c.HALF:(ct + 1) * c.HALF],
                                mT[ct * P:(ct + 1) * P, ho:ho + c.HALF])
                        for t in range(c.T):
                            tb, tt = t // c.TB, t % c.TB
                            for ps in range(c.PS1):
                                hst = hstp.tile([P, c.CT * P], BF, tag="hst")
                                for r in range(c.NC):
                                    ro = (r * c.TB * c.PS1 + tt * c.PS1 + ps) * c.NOWN
                                    nc.sync.dma_start(
                                        hst[:, r * c.NT * P:(r + 1) * c.NT * P].rearrange(
                                            "p (nt f) -> p nt f", f=P),
                                        ag_outs[tb][ro:ro + c.NOWN, :].rearrange(
                                            "(nt p) f -> p nt f", p=P))
                                for (fo, fl) in FCH:
                                    pa = aps.tile([P, fl], F32, tag="pa")
                                    for ct in range(c.CT):
                                        nc.tensor.matmul(
                                            pa[:], hst[:, ct * P:(ct + 1) * P],
                                            mt_sb[:, ct * c.HALF + fo:ct * c.HALF + fo + fl],
                                            start=(ct == 0), stop=(ct == c.CT - 1))
                                    stg = astg.tile([P, fl], BF, tag="astg")
                                    nc.scalar.activation(stg[:], pa[:], Relu,
                                                         bias=cb1_sb[:, ps:ps + 1])
                                    nc.sync.dma_start(agg1[t, ps, :, ho + fo:ho + fo + fl], stg[:])

                # ================= Phase 3: GRU layer 2 (gi2 from streamed agg1) =====
                with tc.tile_pool(name="p3", bufs=1) as p3, \
                     tc.tile_pool(name="p3x", bufs=3) as p3x, \
                     tc.tile_pool(name="p3w", bufs=2) as p3w, \
                     tc.tile_pool(name="ps3", bufs=5, space="PSUM") as ps3, \
                     tc.tile_pool(name="ps3t", bufs=2, space="PSUM") as ps3t, \
                     tc.tile_pool(name="p3s", bufs=2) as p3s:
                    h2 = p3.tile([P, c.PS2 * c.NOWN], BF)
                    nc.vector.memset(h2[:], 0.0)
                    for t in range(c.T):
                        a1 = p3x.tile([P, c.PS1 * c.NOWN], BF, tag="a1")
                        for ps in range(c.PS1):
                            nc.sync.dma_start(a1[:, ps * c.NOWN:(ps + 1) * c.NOWN], agg1[t, ps])
                        gru_step([a1[:, ps * c.NOWN:(ps + 1) * c.NOWN] for ps in range(c.PS1)],
                                 list(range(c.PS1)), wih2_sb, whh2_sb, b2rz_sb, b2hn_sb,
                                 b2in_sb, h2, c.PS2, ps3, p3w, c.G2)
                    # transpose h2[T-1] to node-major, all-gather
                    for ps in range(c.PS2):
                        stg = p3s.tile([P, c.NOWN], BF, tag="stg2")
                        for nt in range(c.NT):
                            pt = ps3t.tile([P, P], BF, tag="ps_t2")
                            nc.tensor.transpose(
                                pt[:], h2[:, ps * c.NOWN + nt * P:ps * c.NOWN + (nt + 1) * P],
                                ident_sb[:])
                            nc.vector.tensor_copy(stg[:, nt * P:(nt + 1) * P], pt[:])
                        nc.sync.dma_start(
                            ag2_in[:].rearrange("(nt p) f -> p nt f", p=P),
                            stg[:].rearrange("p (nt f) -> p nt f", f=P))
                    nc.gpsimd.collective_compute(
                        "AllGather", mybir.AluOpType.bypass, replica_groups=rg,
                        ins=[ag2_in.opt()], outs=[ag2_out.opt()])

                # ================= Phase 4: agg2 + final linear + softmax ============
                with tc.tile_pool(name="p4", bufs=1) as p4, \
                     tc.tile_pool(name="p4m", bufs=2) as p4m, \
                     tc.tile_pool(name="p4h", bufs=1) as p4h, \
                     tc.tile_pool(name="ps4", bufs=4, space="PSUM") as ps4, \
                     tc.tile_pool(name="p4w", bufs=2) as p4w:
                    h2g = p4h.tile([P, c.CT * P], BF)
                    for r in range(c.NC):
                        nc.sync.dma_start(
                            h2g[:, r * c.NT * P:(r + 1) * c.NT * P].rearrange(
                                "p (nt f) -> p nt f", f=P),
                            ag2_out[r * c.NOWN:(r + 1) * c.NOWN, :].rearrange(
                                "(nt p) f -> p nt f", p=P))
                    out2T = p4.tile([P, c.NOWN], BF)
                    for (fo, fl) in fchunks(c.NOWN, 256):
                        mt2 = p4m.tile([P, c.CT * 256], BF, tag="mt2")
                        for ct in range(c.CT):
                            nc.sync.dma_start(mt2[:, ct * fl:(ct + 1) * fl],
                                              mT[ct * P:(ct + 1) * P, fo:fo + fl])
                        pa = ps4.tile([P, fl], F32, tag="pa2", bufs=2)
                        for ct in range(c.CT):
                            nc.tensor.matmul(pa[:], h2g[:, ct * P:(ct + 1) * P],
                                             mt2[:, ct * fl:ct * fl + fl],
                                             start=(ct == 0), stop=(ct == c.CT - 1))
                        nc.scalar.activation(out2T[:, fo:fo + fl], pa[:], Iden,
                                             bias=cb2_sb[:, 0:1])
                    # final linear: logits_partial[j] = sum_n out2[n,:] . linW[j, n, :]
                    plog = ps4.tile([1, c.OUT], F32, tag="plog", bufs=1)
                    for nt in range(c.NT):
                        lw = p4w.tile([P, P * c.OUT], BF, tag="lw")
                        nc.sync.dma_start(lw[:].rearrange("p (n j) -> p n j", j=c.OUT),
                                          linW4[nt])
                        for ni in range(P):
                            n = nt * P + ni
                            nc.tensor.matmul(plog[:], out2T[:, n:n + 1],
                                             lw[:, ni * c.OUT:(ni + 1) * c.OUT],
                                             start=(nt == 0 and ni == 0),
                                             stop=(nt == c.NT - 1 and ni == P - 1))
                    lpart = p4.tile([1, c.OUT], F32)
                    nc.scalar.activation(lpart[:], plog[:], Copy)
                    nc.sync.dma_start(ar_in[:], lpart[:])
                    nc.gpsimd.collective_compute(
                        "AllReduce", mybir.AluOpType.add, replica_groups=rg,
                        ins=[ar_in.opt()], outs=[ar_out.opt()])
                    lg = p4.tile([1, c.OUT], F32)
                    nc.sync.dma_start(lg[:], ar_out[:])
                    nc.vector.tensor_add(lg[:], lg[:], linb_sb[:])
                    mx = p4.tile([1, 1], F32)
                    nc.vector.tensor_reduce(mx[:], lg[:], mybir.AxisListType.X,
                                            mybir.AluOpType.max, negate=True)
                    ex = p4.tile([1, c.OUT], F32)
                    nc.scalar.activation(ex[:], lg[:], Exp, bias=mx[:, 0:1])
                    sm = p4.tile([1, 1], F32)
                    nc.vector.tensor_reduce(sm[:], ex[:], mybir.AxisListType.X,
                                            mybir.AluOpType.add)
                    rcp = p4.tile([1, 1], F32)
                    nc.vector.reciprocal(rcp[:], sm[:])
                    res = p4.tile([1, c.OUT], F32)
                    nc.vector.tensor_scalar_mul(res[:], ex[:], rcp[:, 0:1])
                    nc.sync.dma_start(out[:], res[:])

    nc.compile()
    return nc


def host_prep(cfg: Cfg, x, edge_index, W_ih1, W_hh1, b_ih1, b_hh1, bias1,
              W_ih2, W_hh2, b_ih2, b_hh2, bias2, lin_W, lin_b):
    """Shard + preprocess FULL inputs into per-core in_maps."""
    c = cfg
    x = np.asarray(x, np.float32)
    edge_index = np.asarray(edge_index)
    # dense normalized adjacency, padded: M[dst, src]
    row, col = edge_index[0], edge_index[1]
    loops = np.arange(c.N, dtype=row.dtype)
    row = np.concatenate([row, loops])
    col = np.concatenate([col, loops])
    deg = np.zeros(c.N, np.float32)
    np.add.at(deg, col, 1.0)
    dis = np.where(deg > 0, deg ** -0.5, 0.0).astype(np.float32)
    norm = dis[row] * dis[col]
    M = np.zeros((c.NPAD, c.NPAD), np.float32)
    np.add.at(M, (col, row), norm)

    xp = np.zeros((c.T, c.NPAD, c.DIN), np.float32)
    xp[:, :c.N, :] = x

    def col_f32(v):
        return np.asarray(v, np.float32).reshape(-1, 1)

    W_ih1 = np.asarray(W_ih1, np.float32); W_hh1 = np.asarray(W_hh1, np.float32)
    W_ih2 = np.asarray(W_ih2, np.float32); W_hh2 = np.asarray(W_hh2, np.float32)
    b_ih1 = np.asarray(b_ih1, np.float32); b_hh1 = np.asarray(b_hh1, np.float32)
    b_ih2 = np.asarray(b_ih2, np.float32); b_hh2 = np.asarray(b_hh2, np.float32)
    lin_W = np.asarray(lin_W, np.float32)

    common = dict(
        wih1T=W_ih1.T.astype(BF16), whh1T=W_hh1.T.astype(BF16),
        wih2T=W_ih2.T.astype(BF16), whh2T=W_hh2.T.astype(BF16),
        b1_rz=col_f32((b_ih1 + b_hh1)[:2 * c.H1]), b1_hn=col_f32(b_hh1[2 * c.H1:]),
        b1_in=col_f32(b_ih1[2 * c.H1:]),
        b2_rz=col_f32((b_ih2 + b_hh2)[:2 * c.H2]), b2_hn=col_f32(b_hh2[2 * c.H2:]),
        b2_in=col_f32(b_ih2[2 * c.H2:]),
        cb1=col_f32(bias1), cb2=col_f32(bias2),
        linb=np.asarray(lin_b, np.float32).reshape(1, c.OUT),
        ident=np.eye(P, dtype=BF16),
    )
    # lin_W: (OUT, N*H2) -> (OUT, NPAD, H2) padded
    lw = np.zeros((c.OUT, c.NPAD, c.H2), np.float32)
    lw[:, :c.N, :] = lin_W.reshape(c.OUT, c.N, c.H2)

    in_maps = []
    for k in range(c.NC):
        sl = slice(k * c.NOWN, (k + 1) * c.NOWN)
        m = dict(common)
        m["xT"] = np.ascontiguousarray(xp[:, sl, :].transpose(0, 2, 1)).astype(BF16)
        m["mT"] = np.ascontiguousarray(M[sl, :].T).astype(BF16)
        # (OUT, NOWN, H2) -> (NT, H2, P, OUT)
        lwk = lw[:, sl, :].reshape(c.OUT, c.NT, P, c.H2)
        m["linW4"] = np.ascontiguousarray(lwk.transpose(1, 3, 2, 0)).astype(BF16)
        in_maps.append(m)
    return in_maps


_CACHE = {}


def _get_built(key, cfg):
    if key not in _CACHE:
        _CACHE[key] = build(cfg)
    return _CACHE[key]


def run(cfg: Cfg, inputs, trace=False):
    _install_ntff_hook()
    nc = _get_built(("cfg", cfg.T, cfg.N), cfg)
    in_maps = host_prep(cfg, **inputs)
    res = run_bass_kernel_spmd(nc, in_maps, core_ids=list(range(cfg.NC)), trace=trace)
    return res


def kernel(**inputs) -> np.ndarray:
    cfg = Cfg()
    res = run(cfg, inputs)
    return np.asarray(res.results[0]["out"], np.float32)
